# revision 51
# baseline (speedup 1.0000x reference)
"""BertSelfAttention TRN2 Bass kernel (8-core data-parallel over batch).

Per core (one batch element), per head:
  qk projection -> q,k in SBUF (biases folded in via rank-1 ones-row matmuls)
  stats pass:  scores in [q-part, k-free] orientation (mask folded via aux
               row) -> per-query max m_q via DVE reduce_max
  main pass:   scores in [k-part, q-free] orientation with two aux rows
               (mask, and -1 paired against a c-row holding +m_q, so the
               matmul itself computes s - m) -> single exp on ACT -> bf16
               probs e8
  context:     out[q, d] orientation: stationary e8 [k,q-tile], moving
               v_aug [k, d|1] (bf16) accumulated over k tiles; the 65th
               column of v_aug is 1 so the same matmul yields Z; normalize
               by 1/Z straight out of PSUM (DVE recip + ACT copy-scale).

The per-head work is emitted as a 4-stage software pipeline
(proj h | stats h-1 | scores+exp h-2 | context+normalize h-3) with the
stages' instructions interleaved step-by-step so the in-order engine
queues (PE / DVE / ACT) always have independent work.
All f32 matmuls run in float32r (fast PE mode, fp32 PSUM accumulation).
"""
import sys

sys.path.insert(0, "/opt/trn_rl_repo")

import numpy as np
import concourse.bacc as bacc
import concourse.mybir as mybir
import concourse.tile as tile
from concourse.bass_utils import run_bass_kernel_spmd

F32 = mybir.dt.float32
F32R = mybir.dt.float32r
BF16 = mybir.dt.bfloat16
EXP = mybir.ActivationFunctionType.Exp
COPY = mybir.ActivationFunctionType.Copy

HD = 64  # head dim (fixed)
SPREAD_VPROJ = 1

# instruction-name -> logical label, filled during build for profiling
INST_LABELS = {}
_cur_label = [""]


def _label(s):
    _cur_label[0] = s


def _hook_labels(nc):
    orig = nc.get_next_instruction_name

    def wrapped():
        name = orig()
        INST_LABELS[name] = _cur_label[0]
        return name

    nc.get_next_instruction_name = wrapped


def build_module(T, H, NH):
    """One-core program; run SPMD on 8 cores with per-core batch slices."""
    NT = T // 128      # token tiles
    NHT = H // 128     # hidden-dim tiles
    QC = min(512, T)   # moving chunk (>=256 keeps f32r at full rate)
    NQC = T // QC

    nc = bacc.Bacc("TRN2", target_bir_lowering=False, debug=False, num_devices=8)
    _hook_labels(nc)

    hidden = nc.dram_tensor("hidden", [T, H], F32R, kind="ExternalInput").ap()
    w = nc.dram_tensor("w", [H, 3 * H], F32R, kind="ExternalInput").ap()
    mask_neg = nc.dram_tensor("mask_neg", [1, T], F32R, kind="ExternalInput").ap()
    ones_row = nc.dram_tensor("ones_row", [1, T], F32R, kind="ExternalInput").ap()
    neg_row = nc.dram_tensor("neg_row", [1, T], F32R, kind="ExternalInput").ap()
    qkb = nc.dram_tensor("qkb", [1, 128 * NH], F32R, kind="ExternalInput").ap()
    vbat = nc.dram_tensor("vbat", [1, (HD + 1) * NH], F32R, kind="ExternalInput").ap()
    wv_pre = nc.dram_tensor(
        "wv_pre", [128, NHT * NH * (HD + 1)], F32R, kind="ExternalInput"
    ).ap()
    ident_r = nc.dram_tensor("ident_r", [128, 128], F32R, kind="ExternalInput").ap()
    out = nc.dram_tensor("out", [T, H], F32, kind="ExternalOutput").ap()

    out_r = out.rearrange("(qt p) (h d) -> p qt h d", p=128, d=HD)

    with tile.TileContext(nc) as tc:
        with tc.tile_pool(name="persist", bufs=1) as persist, tc.tile_pool(
            name="work", bufs=2
        ) as work, tc.tile_pool(name="e8p", bufs=18) as e8p, tc.tile_pool(
            name="cmp", bufs=2
        ) as cmp, tc.tile_pool(name="outp", bufs=2) as outp, tc.tile_pool(
            name="rzp", bufs=3
        ) as rzp, tc.tile_pool(
            name="psP", bufs=1, space="PSUM"
        ) as psP, tc.tile_pool(name="psS", bufs=1, space="PSUM") as psS, tc.tile_pool(
            name="psE", bufs=1, space="PSUM"
        ) as psE, tc.tile_pool(name="psC", bufs=1, space="PSUM") as psC:
            _label("static")
            # ---- input loads first (hidden tiles gate phase 0); the first
            # v-weight group goes early too so vproj can start promptly ----
            idr = persist.tile([128, 128], F32R, tag="idr")
            nc.sync.dma_start(out=idr, in_=ident_r)
            GH = min(4, NH)    # heads per v-projection chunk (moving >= 256)
            NG = NH // GH
            # v weights arrive prepacked+padded (see run_sharded)
            wv = persist.tile([128, NHT, NH, HD + 1], F32R, tag="wv")
            wv_flat = wv.rearrange("p a b c -> p (a b c)")
            WVL = NHT * NH * (HD + 1)

            def emit_wv_dmas(g):
                nc.sync.dma_start(
                    out=wv_flat[:, g * WVL // NG : (g + 1) * WVL // NG],
                    in_=wv_pre[:, g * WVL // NG : (g + 1) * WVL // NG],
                )

            hids = []
            for t in range(NT):
                hid = work.tile([128, H], F32R, tag="hid", bufs=min(3, NT))
                nc.sync.dma_start(out=hid, in_=hidden[t * 128 : (t + 1) * 128, :])
                hids.append(hid)
                if t == NT // 2 - 1:
                    emit_wv_dmas(0)
            # ---- static constants ----
            onesr = persist.tile([1, T], F32R, tag="onesr")
            nc.sync.dma_start(out=onesr, in_=ones_row)
            qkbt = persist.tile([1, 128 * NH], F32R, tag="qkbt")
            nc.sync.dma_start(out=qkbt, in_=qkb)
            vbat_t = persist.tile([1, (HD + 1) * NH], F32R, tag="vbat_t")
            nc.sync.dma_start(out=vbat_t, in_=vbat)

            # triple-buffered q/k aux buffers (head h uses slot h%3):
            # qaux rows: 0:64 q, 64 ones (static), 65 c=+max (per head)
            # kaux rows: 0:64 k, 64 mask*-1e4 (static), 65 -1 (static)
            qauxs, kauxs = [], []
            for s in range(4):
                qa = persist.tile([66, T], F32R, tag=f"qaux{s}")
                ka = persist.tile([66, T], F32R, tag=f"kaux{s}")
                nc.sync.dma_start(out=qa[64:65, :], in_=ones_row)
                nc.sync.dma_start(out=ka[64:65, :], in_=mask_neg)
                nc.sync.dma_start(out=ka[65:66, :], in_=neg_row)
                qauxs.append(qa)
                kauxs.append(ka)

            # ---- phase 0: hT[p, ht, t] = hidden[t, ht*128+p] ----
            hT = persist.tile([128, NHT, T], F32R, tag="hT")
            for t in range(NT):
                _label(f"ph0:t{t}")
                pool = psS if t % 2 == 0 else psP
                xpt = pool.tile([128, NHT, 128], F32R, tag="t")
                for hb in range(NHT):
                    nc.tensor.transpose(
                        xpt[:, hb, :], hids[t][:, hb * 128 : (hb + 1) * 128], idr[:]
                    )
                if t % 2 == 0:
                    nc.vector.tensor_copy(hT[:, :, t * 128 : (t + 1) * 128], xpt[:])
                else:
                    nc.scalar.copy(hT[:, :, t * 128 : (t + 1) * 128], xpt[:])

            # ---- phase 1 setup: remaining v-weight chunks ----
            for g in range(1, NG):
                emit_wv_dmas(g)
            v_aug = persist.tile([128, NT, NH, HD + 1], BF16, tag="v_aug")

            def vproj_chunk(t, g):
                """v_aug[:, t, g*GH:(g+1)*GH, :]: v proj + bias, with the
                augmentation 1s landing via the (start=True) bias matmul, so
                the whole destination region has a single DVE writer."""
                _label(f"ph1:t{t}.{g}")
                vp = psC.tile([128, GH, HD + 1], F32, tag="cq", name="vp")
                # bias+ones first: resets the full region incl the 65th col
                nc.tensor.matmul(
                    vp[:],
                    onesr[0:1, t * 128 : (t + 1) * 128],
                    vbat_t[0:1, g * GH * (HD + 1) : (g + 1) * GH * (HD + 1)],
                    start=True,
                    stop=False,
                )
                for ht in range(NHT):
                    nc.tensor.matmul(
                        vp[:],
                        hT[:, ht, t * 128 : (t + 1) * 128],
                        wv[:, ht, g * GH : (g + 1) * GH, :],
                        start=False,
                        stop=(ht == NHT - 1),
                    )
                nc.scalar.copy(v_aug[:, t, g * GH : (g + 1) * GH, :], vp[:])

            vchunks = [(t, g) for g in range(NG) for t in range(NT)]
            if not SPREAD_VPROJ:
                for t, g in vchunks:
                    vproj_chunk(t, g)
                vchunks = []

            # ---- per-head attention: 4-stage software pipeline ----
            state = {}  # head -> dict with live tiles

            def proj_ops(h):
                """Closures emitting the qk projection: per 512-wide half, 8
                accumulating matmuls + bias matmul into a 1-bank psum tile,
                then the q/k copies for that half (q on DVE, k on ACT)."""
                qa, ka = qauxs[h % 4], kauxs[h % 4]
                wqk = work.tile([128, NHT, 128], F32R, tag="wqk")
                nc.sync.dma_start(
                    out=wqk,
                    in_=w[:, h * 3 * HD : h * 3 * HD + 128].rearrange(
                        "(ht p) f -> p ht f", p=128
                    ),
                )
                ops = []
                qkp = psP.tile([128, T], F32, tag="t", name="qkp")

                def mk_half(qc):

                    def mm(ht):
                        _label(f"proj:h{h}")
                        nc.tensor.matmul(
                            qkp[:, qc * QC : (qc + 1) * QC],
                            wqk[:, ht, :],
                            hT[:, ht, qc * QC : (qc + 1) * QC],
                            start=(ht == 0),
                            stop=False,
                        )

                    def bias():
                        _label(f"projb:h{h}")
                        nc.tensor.matmul(
                            qkp[:, qc * QC : (qc + 1) * QC],
                            qkbt[0:1, h * 128 : (h + 1) * 128],
                            onesr[0:1, qc * QC : (qc + 1) * QC],
                            start=False,
                            stop=True,
                        )

                    def copies():
                        _label(f"qkcopy:h{h}")
                        sl = slice(qc * QC, (qc + 1) * QC)
                        nc.vector.tensor_copy(qa[0:64, sl], qkp[0:64, sl])
                        nc.scalar.copy(ka[0:64, sl], qkp[64:128, sl])

                    return [lambda ht=ht: mm(ht) for ht in range(NHT)] + [
                        bias,
                        copies,
                    ]

                for qc in range(NQC):
                    ops.extend(mk_half(qc))
                return ops

            NSTEP = NT  # steps per iteration
            for it in range(NH + 5):
                hP, hS, hE, hC = it, it - 1, it - 3, it - 4

                doP = hP < NH
                doS = 0 <= hS < NH
                doE = 0 <= hE < NH
                doC = 0 <= hC < NH

                pops = []
                if doP:
                    state[hP] = {}
                    pops = proj_ops(hP)
                # spread the v projection over the first four iterations
                # (before the ctx stage starts using the shared psC pool);
                # group g must be complete before ctx of head g*GH at
                # iteration g*GH+4, which this comfortably precedes
                if it < 4 and vchunks and SPREAD_VPROJ:
                    rem = 4 - it
                    nv = (len(vchunks) + rem - 1) // rem
                    for t, g in vchunks[:nv]:
                        pops.append(lambda t=t, g=g: vproj_chunk(t, g))
                    vchunks = vchunks[nv:]
                if doS:
                    stS = state[hS]
                    stS["cmat"] = cmp.tile([128, NT], F32R, tag="cmat", name="cmat")
                if doE:
                    stE = state[hE]
                    stE["e8s"] = []
                if doC:
                    stC = state[hC]
                    stC["ost"] = outp.tile([128, NT, HD], F32, tag="ost", name="ost")
                    stC["ctq"] = psC.tile([128, NT, 128], F32, tag="cq", name="ctq")

                ppos = 0
                nper = (len(pops) + NSTEP - 1) // NSTEP if pops else 0
                for i in range(NSTEP):
                    # pass2(hE) step: scores [k-tile i, all q] -> exp -> bf16
                    if doE:
                        _label(f"pass2:h{hE}.k{i}")
                        qa, ka = qauxs[hE % 4], kauxs[hE % 4]
                        e8 = e8p.tile([128, T], BF16, tag="e8")
                        for qc in range(NQC):
                            sp = psE.tile([128, QC], F32, tag="t", bufs=2)
                            nc.tensor.matmul(
                                sp[:],
                                ka[0:66, i * 128 : (i + 1) * 128],
                                qa[0:66, qc * QC : (qc + 1) * QC],
                                start=True,
                                stop=True,
                            )
                            nc.scalar.activation(
                                out=e8[:, qc * QC : (qc + 1) * QC],
                                in_=sp[:],
                                func=EXP,
                                scale=8.0,
                            )
                        stE["e8s"].append(e8)
                    # stats(hS) step: scores [q-tile i, all k] -> row max
                    if doS:
                        _label(f"stats:h{hS}.q{i}")
                        qa, ka = qauxs[hS % 4], kauxs[hS % 4]
                        smx = psS.tile([128, T], F32, tag="t", name="smx")
                        for qc in range(NQC):
                            nc.tensor.matmul(
                                smx[:, qc * QC : (qc + 1) * QC],
                                qa[0:65, i * 128 : (i + 1) * 128],
                                ka[0:65, qc * QC : (qc + 1) * QC],
                                start=True,
                                stop=True,
                            )
                        nc.vector.reduce_max(
                            stS["cmat"][:, i : i + 1], smx[:],
                            axis=mybir.AxisListType.X,
                        )
                    # proj(hP) chunk
                    for _ in range(nper):
                        if ppos < len(pops):
                            pops[ppos]()
                            ppos += 1
                    # ctx(hC) step: context for q-tile i over all k tiles
                    if doC:
                        _label(f"ctx:h{hC}.q{i}")
                        ctq = stC["ctq"]
                        for kt in range(NT):
                            nc.tensor.matmul(
                                ctq[:, i, 0 : HD + 1],
                                stC["e8s"][kt][:, i * 128 : (i + 1) * 128],
                                v_aug[:, kt, hC, :],
                                start=(kt == 0),
                                stop=(kt == NT - 1),
                            )
                        _label(f"norm:h{hC}.q{i}")
                        rz = rzp.tile([128, 1], F32, tag="rz")
                        nc.vector.reciprocal(rz[:], ctq[:, i, HD : HD + 1])
                        if i % 2 == 0:
                            nc.scalar.activation(
                                out=stC["ost"][:, i, :],
                                in_=ctq[:, i, 0:HD],
                                func=COPY,
                                scale=rz[:],
                            )
                        else:
                            nc.vector.tensor_scalar_mul(
                                stC["ost"][:, i, :], ctq[:, i, 0:HD], rz[:]
                            )

                # emit any leftover proj ops
                while ppos < len(pops):
                    pops[ppos]()
                    ppos += 1

                # stats(hS) epilogue: c row -> qaux[65]
                if doS:
                    _label(f"crow:h{hS}")
                    ctile = psS.tile([NT, 128], F32R, tag="t", name="ctile")
                    nc.tensor.transpose(ctile[:], stS["cmat"][:], idr[:])
                    ctr = work.tile([NT, 128], F32R, tag="ctr")
                    nc.vector.tensor_copy(ctr[:], ctile[:])
                    nc.sync.dma_start(out=qauxs[hS % 4][65:66, :], in_=ctr[:])

                # ctx(hC) epilogue: store
                if doC:
                    _label(f"ostore:h{hC}")
                    nc.sync.dma_start(out=out_r[:, :, hC, :], in_=stC["ost"])
                    del state[hC]

    nc.compile()
    return nc


_module_cache = {}


def _get_module(T, H, NH):
    key = (T, H, NH)
    if key not in _module_cache:
        _module_cache[key] = build_module(T, H, NH)
    return _module_cache[key]


def run_sharded(hidden_states, attention_mask, w_qkv, b_qkv, trace=False):
    B, T, H = hidden_states.shape
    NH = H // HD
    nc = _get_module(T, H, NH)

    w_np = np.ascontiguousarray(w_qkv.astype(np.float32))
    b_np = np.asarray(b_qkv, dtype=np.float32)
    # qkb[h*128 + p] = b[h*192 + p]  (q bias 0:64, k bias 64:128 per head)
    qkb = np.empty((1, 128 * NH), np.float32)
    vbat = np.ones((1, (HD + 1) * NH), np.float32)
    for h in range(NH):
        qkb[0, h * 128 : (h + 1) * 128] = b_np[h * 3 * HD : h * 3 * HD + 128]
        vbat[0, h * (HD + 1) : h * (HD + 1) + HD] = b_np[
            h * 3 * HD + 2 * HD : h * 3 * HD + 3 * HD
        ]
    NHT = H // 128
    # wv_pre[p, ht, h, 0:64] = w[ht*128+p, h*192+128 : h*192+192]; slot 64 = 0
    wv4 = w_np.reshape(NHT, 128, NH, 3, HD)[:, :, :, 2, :]   # [ht, p, h, d]
    wv_pre = np.zeros((NHT, 128, NH, HD + 1), np.float32)
    wv_pre[:, :, :, 0:HD] = wv4
    wv_pre = np.ascontiguousarray(
        wv_pre.transpose(1, 0, 2, 3).reshape(128, NHT * NH * (HD + 1))
    )
    ones_row = np.ones((1, T), np.float32)
    neg_row = np.full((1, T), -1.0, np.float32)
    ident = np.eye(128, dtype=np.float32)

    in_maps = []
    for b in range(B):
        m = np.asarray(attention_mask[b]).reshape(-1).astype(np.float32)
        in_maps.append(
            dict(
                hidden=np.ascontiguousarray(hidden_states[b].astype(np.float32)),
                w=w_np,
                mask_neg=(m * np.float32(-10000.0)).reshape(1, T),
                ones_row=ones_row,
                neg_row=neg_row,
                qkb=qkb,
                vbat=vbat,
                wv_pre=wv_pre,
                ident_r=ident,
            )
        )
    res = run_bass_kernel_spmd(nc, in_maps, core_ids=list(range(B)), trace=trace)
    return np.stack([res.results[b]["out"] for b in range(B)]), res


def kernel(hidden_states, attention_mask, w_qkv, b_qkv):
    out, _ = run_sharded(
        np.asarray(hidden_states),
        np.asarray(attention_mask),
        np.asarray(w_qkv),
        np.asarray(b_qkv),
    )
    return out.astype(np.float32)


# revision 52
# speedup vs baseline: 1.0044x; 1.0044x over previous
"""BertSelfAttention TRN2 Bass kernel (8-core data-parallel over batch).

Per core (one batch element), per head:
  qk projection -> q,k in SBUF (biases folded in via rank-1 ones-row matmuls)
  stats pass:  scores in [q-part, k-free] orientation (mask folded via aux
               row) -> per-query max m_q via DVE reduce_max
  main pass:   scores in [k-part, q-free] orientation with two aux rows
               (mask, and -1 paired against a c-row holding +m_q, so the
               matmul itself computes s - m) -> single exp on ACT -> bf16
               probs e8
  context:     out[q, d] orientation: stationary e8 [k,q-tile], moving
               v_aug [k, d|1] (bf16) accumulated over k tiles; the 65th
               column of v_aug is 1 so the same matmul yields Z; normalize
               by 1/Z straight out of PSUM (DVE recip + ACT copy-scale).

The per-head work is emitted as a 4-stage software pipeline
(proj h | stats h-1 | scores+exp h-2 | context+normalize h-3) with the
stages' instructions interleaved step-by-step so the in-order engine
queues (PE / DVE / ACT) always have independent work.
All f32 matmuls run in float32r (fast PE mode, fp32 PSUM accumulation).
"""
import sys

sys.path.insert(0, "/opt/trn_rl_repo")

import numpy as np
import concourse.bacc as bacc
import concourse.mybir as mybir
import concourse.tile as tile
from concourse.bass_utils import run_bass_kernel_spmd

F32 = mybir.dt.float32
F32R = mybir.dt.float32r
BF16 = mybir.dt.bfloat16
EXP = mybir.ActivationFunctionType.Exp
COPY = mybir.ActivationFunctionType.Copy

HD = 64  # head dim (fixed)
SPREAD_VPROJ = 1

# instruction-name -> logical label, filled during build for profiling
INST_LABELS = {}
_cur_label = [""]


def _label(s):
    _cur_label[0] = s


def _hook_labels(nc):
    orig = nc.get_next_instruction_name

    def wrapped():
        name = orig()
        INST_LABELS[name] = _cur_label[0]
        return name

    nc.get_next_instruction_name = wrapped


def build_module(T, H, NH):
    """One-core program; run SPMD on 8 cores with per-core batch slices."""
    NT = T // 128      # token tiles
    NHT = H // 128     # hidden-dim tiles
    QC = min(512, T)   # moving chunk (>=256 keeps f32r at full rate)
    NQC = T // QC

    nc = bacc.Bacc("TRN2", target_bir_lowering=False, debug=False, num_devices=8)
    _hook_labels(nc)

    hidden = nc.dram_tensor("hidden", [T, H], F32R, kind="ExternalInput").ap()
    w = nc.dram_tensor("w", [H, 3 * H], F32R, kind="ExternalInput").ap()
    mask_neg = nc.dram_tensor("mask_neg", [1, T], F32R, kind="ExternalInput").ap()
    ones_row = nc.dram_tensor("ones_row", [1, T], F32R, kind="ExternalInput").ap()
    neg_row = nc.dram_tensor("neg_row", [1, T], F32R, kind="ExternalInput").ap()
    qkb = nc.dram_tensor("qkb", [1, 128 * NH], F32R, kind="ExternalInput").ap()
    vbat = nc.dram_tensor("vbat", [1, (HD + 1) * NH], F32R, kind="ExternalInput").ap()
    wv_pre = nc.dram_tensor(
        "wv_pre", [128, NHT * NH * (HD + 1)], F32R, kind="ExternalInput"
    ).ap()
    ident_r = nc.dram_tensor("ident_r", [128, 128], F32R, kind="ExternalInput").ap()
    out = nc.dram_tensor("out", [T, H], F32, kind="ExternalOutput").ap()

    out_r = out.rearrange("(qt p) (h d) -> p qt h d", p=128, d=HD)

    with tile.TileContext(nc) as tc:
        with tc.tile_pool(name="persist", bufs=1) as persist, tc.tile_pool(
            name="work", bufs=2
        ) as work, tc.tile_pool(name="e8p", bufs=18) as e8p, tc.tile_pool(
            name="cmp", bufs=2
        ) as cmp, tc.tile_pool(name="outp", bufs=2) as outp, tc.tile_pool(
            name="rzp", bufs=3
        ) as rzp, tc.tile_pool(
            name="psP", bufs=1, space="PSUM"
        ) as psP, tc.tile_pool(name="psS", bufs=1, space="PSUM") as psS, tc.tile_pool(
            name="psE", bufs=1, space="PSUM"
        ) as psE, tc.tile_pool(name="psC", bufs=1, space="PSUM") as psC:
            _label("static")
            # ---- input loads first (hidden tiles gate phase 0); the first
            # v-weight group goes early too so vproj can start promptly ----
            idr = persist.tile([128, 128], F32R, tag="idr")
            nc.sync.dma_start(out=idr, in_=ident_r)
            GH = min(4, NH)    # heads per v-projection chunk (moving >= 256)
            NG = NH // GH
            # v weights arrive prepacked+padded (see run_sharded)
            wv = persist.tile([128, NHT, NH, HD + 1], F32R, tag="wv")
            wv_flat = wv.rearrange("p a b c -> p (a b c)")
            WVL = NHT * NH * (HD + 1)

            def emit_wv_dmas(g):
                nc.sync.dma_start(
                    out=wv_flat[:, g * WVL // NG : (g + 1) * WVL // NG],
                    in_=wv_pre[:, g * WVL // NG : (g + 1) * WVL // NG],
                )

            hids = []
            for t in range(NT):
                hid = work.tile([128, H], F32R, tag="hid", bufs=min(3, NT))
                nc.sync.dma_start(out=hid, in_=hidden[t * 128 : (t + 1) * 128, :])
                hids.append(hid)
                if t == NT // 2 - 1:
                    emit_wv_dmas(0)
            # ---- static constants ----
            onesr = persist.tile([1, T], F32R, tag="onesr")
            nc.sync.dma_start(out=onesr, in_=ones_row)
            qkbt = persist.tile([1, 128 * NH], F32R, tag="qkbt")
            nc.sync.dma_start(out=qkbt, in_=qkb)
            vbat_t = persist.tile([1, (HD + 1) * NH], F32R, tag="vbat_t")
            nc.sync.dma_start(out=vbat_t, in_=vbat)

            # triple-buffered q/k aux buffers (head h uses slot h%3):
            # qaux rows: 0:64 q, 64 ones (static), 65 c=+max (per head)
            # kaux rows: 0:64 k, 64 mask*-1e4 (static), 65 -1 (static)
            qauxs, kauxs = [], []
            for s in range(4):
                qa = persist.tile([66, T], F32R, tag=f"qaux{s}")
                ka = persist.tile([66, T], F32R, tag=f"kaux{s}")
                nc.sync.dma_start(out=qa[64:65, :], in_=ones_row)
                nc.sync.dma_start(out=ka[64:65, :], in_=mask_neg)
                nc.sync.dma_start(out=ka[65:66, :], in_=neg_row)
                qauxs.append(qa)
                kauxs.append(ka)

            # ---- phase 0: hT[p, ht, t] = hidden[t, ht*128+p] ----
            hT = persist.tile([128, NHT, T], F32R, tag="hT")
            for t in range(NT):
                _label(f"ph0:t{t}")
                pool = psS if t % 2 == 0 else psP
                xpt = pool.tile([128, NHT, 128], F32R, tag="t")
                for hb in range(NHT):
                    nc.tensor.transpose(
                        xpt[:, hb, :], hids[t][:, hb * 128 : (hb + 1) * 128], idr[:]
                    )
                if t % 2 == 0:
                    nc.vector.tensor_copy(hT[:, :, t * 128 : (t + 1) * 128], xpt[:])
                else:
                    nc.scalar.copy(hT[:, :, t * 128 : (t + 1) * 128], xpt[:])

            # ---- phase 1 setup: remaining v-weight chunks ----
            for g in range(1, NG):
                emit_wv_dmas(g)
            v_aug = persist.tile([128, NT, NH, HD + 1], BF16, tag="v_aug")

            def vproj_chunk(t, g):
                """v_aug[:, t, g*GH:(g+1)*GH, :]: v proj + bias, with the
                augmentation 1s landing via the (start=True) bias matmul, so
                the whole destination region has a single DVE writer."""
                _label(f"ph1:t{t}.{g}")
                vp = psC.tile([128, GH, HD + 1], F32, tag="cq", name="vp")
                # bias+ones first: resets the full region incl the 65th col
                nc.tensor.matmul(
                    vp[:],
                    onesr[0:1, t * 128 : (t + 1) * 128],
                    vbat_t[0:1, g * GH * (HD + 1) : (g + 1) * GH * (HD + 1)],
                    start=True,
                    stop=False,
                )
                for ht in range(NHT):
                    nc.tensor.matmul(
                        vp[:],
                        hT[:, ht, t * 128 : (t + 1) * 128],
                        wv[:, ht, g * GH : (g + 1) * GH, :],
                        start=False,
                        stop=(ht == NHT - 1),
                    )
                nc.scalar.copy(v_aug[:, t, g * GH : (g + 1) * GH, :], vp[:])

            vchunks = [(t, g) for g in range(NG) for t in range(NT)]
            if not SPREAD_VPROJ:
                for t, g in vchunks:
                    vproj_chunk(t, g)
                vchunks = []

            # ---- per-head attention: 4-stage software pipeline ----
            state = {}  # head -> dict with live tiles

            def proj_ops(h):
                """Closures emitting the qk projection: per 512-wide half, 8
                accumulating matmuls + bias matmul into a 1-bank psum tile,
                then the q/k copies for that half (q on DVE, k on ACT)."""
                qa, ka = qauxs[h % 4], kauxs[h % 4]
                wqk = work.tile([128, NHT, 128], F32R, tag="wqk")
                nc.sync.dma_start(
                    out=wqk,
                    in_=w[:, h * 3 * HD : h * 3 * HD + 128].rearrange(
                        "(ht p) f -> p ht f", p=128
                    ),
                )
                ops = []
                qkp = psP.tile([128, T], F32, tag="t", name="qkp")

                def mk_half(qc):

                    def mm(ht):
                        _label(f"proj:h{h}")
                        nc.tensor.matmul(
                            qkp[:, qc * QC : (qc + 1) * QC],
                            wqk[:, ht, :],
                            hT[:, ht, qc * QC : (qc + 1) * QC],
                            start=(ht == 0),
                            stop=False,
                        )

                    def bias():
                        _label(f"projb:h{h}")
                        nc.tensor.matmul(
                            qkp[:, qc * QC : (qc + 1) * QC],
                            qkbt[0:1, h * 128 : (h + 1) * 128],
                            onesr[0:1, qc * QC : (qc + 1) * QC],
                            start=False,
                            stop=True,
                        )

                    def copies():
                        _label(f"qkcopy:h{h}")
                        sl = slice(qc * QC, (qc + 1) * QC)
                        nc.vector.tensor_copy(qa[0:64, sl], qkp[0:64, sl])
                        nc.scalar.copy(ka[0:64, sl], qkp[64:128, sl])

                    return [lambda ht=ht: mm(ht) for ht in range(NHT)] + [
                        bias,
                        copies,
                    ]

                for qc in range(NQC):
                    ops.extend(mk_half(qc))
                return ops

            NSTEP = NT  # steps per iteration
            for it in range(NH + 5):
                hP, hS, hE, hC = it, it - 1, it - 3, it - 4

                doP = hP < NH
                doS = 0 <= hS < NH
                doE = 0 <= hE < NH
                doC = 0 <= hC < NH

                pops = []
                if doP:
                    state[hP] = {}
                    pops = proj_ops(hP)
                # spread the v projection over the first four iterations
                # (before the ctx stage starts using the shared psC pool);
                # group g must be complete before ctx of head g*GH at
                # iteration g*GH+4, which this comfortably precedes
                if it < 4 and vchunks and SPREAD_VPROJ:
                    rem = 4 - it
                    nv = (len(vchunks) + rem - 1) // rem
                    for t, g in vchunks[:nv]:
                        pops.append(lambda t=t, g=g: vproj_chunk(t, g))
                    vchunks = vchunks[nv:]
                if doS:
                    stS = state[hS]
                    stS["cmat"] = cmp.tile([128, NT], F32R, tag="cmat", name="cmat")
                if doE:
                    stE = state[hE]
                    stE["e8s"] = []
                if doC:
                    stC = state[hC]
                    stC["ost"] = outp.tile([128, NT, HD], F32, tag="ost", name="ost")
                    stC["ctq"] = psC.tile([128, NT, 128], F32, tag="cq", name="ctq")

                ppos = 0
                nper = (len(pops) + NSTEP - 1) // NSTEP if pops else 0
                for i in range(NSTEP):
                    # stats(hS) step: scores [q-tile i, all k] -> row max
                    if doS:
                        _label(f"stats:h{hS}.q{i}")
                        qa, ka = qauxs[hS % 4], kauxs[hS % 4]
                        smx = psS.tile([128, T], F32, tag="t", name="smx")
                        for qc in range(NQC):
                            nc.tensor.matmul(
                                smx[:, qc * QC : (qc + 1) * QC],
                                qa[0:65, i * 128 : (i + 1) * 128],
                                ka[0:65, qc * QC : (qc + 1) * QC],
                                start=True,
                                stop=True,
                            )
                        nc.vector.reduce_max(
                            stS["cmat"][:, i : i + 1], smx[:],
                            axis=mybir.AxisListType.X,
                        )
                    # proj(hP) chunk
                    for _ in range(nper):
                        if ppos < len(pops):
                            pops[ppos]()
                            ppos += 1
                    # pass2(hE) step: scores [k-tile i, all q] -> exp -> bf16
                    if doE:
                        _label(f"pass2:h{hE}.k{i}")
                        qa, ka = qauxs[hE % 4], kauxs[hE % 4]
                        e8 = e8p.tile([128, T], BF16, tag="e8")
                        for qc in range(NQC):
                            sp = psE.tile([128, QC], F32, tag="t", bufs=2)
                            nc.tensor.matmul(
                                sp[:],
                                ka[0:66, i * 128 : (i + 1) * 128],
                                qa[0:66, qc * QC : (qc + 1) * QC],
                                start=True,
                                stop=True,
                            )
                            nc.scalar.activation(
                                out=e8[:, qc * QC : (qc + 1) * QC],
                                in_=sp[:],
                                func=EXP,
                                scale=8.0,
                            )
                        stE["e8s"].append(e8)
                    # ctx(hC) step: context for q-tile i over all k tiles
                    if doC:
                        _label(f"ctx:h{hC}.q{i}")
                        ctq = stC["ctq"]
                        for kt in range(NT):
                            nc.tensor.matmul(
                                ctq[:, i, 0 : HD + 1],
                                stC["e8s"][kt][:, i * 128 : (i + 1) * 128],
                                v_aug[:, kt, hC, :],
                                start=(kt == 0),
                                stop=(kt == NT - 1),
                            )
                        _label(f"norm:h{hC}.q{i}")
                        rz = rzp.tile([128, 1], F32, tag="rz")
                        nc.vector.reciprocal(rz[:], ctq[:, i, HD : HD + 1])
                        if i % 2 == 0:
                            nc.scalar.activation(
                                out=stC["ost"][:, i, :],
                                in_=ctq[:, i, 0:HD],
                                func=COPY,
                                scale=rz[:],
                            )
                        else:
                            nc.vector.tensor_scalar_mul(
                                stC["ost"][:, i, :], ctq[:, i, 0:HD], rz[:]
                            )

                # emit any leftover proj ops
                while ppos < len(pops):
                    pops[ppos]()
                    ppos += 1

                # stats(hS) epilogue: c row -> qaux[65]
                if doS:
                    _label(f"crow:h{hS}")
                    ctile = psS.tile([NT, 128], F32R, tag="t", name="ctile")
                    nc.tensor.transpose(ctile[:], stS["cmat"][:], idr[:])
                    ctr = work.tile([NT, 128], F32R, tag="ctr")
                    nc.vector.tensor_copy(ctr[:], ctile[:])
                    nc.sync.dma_start(out=qauxs[hS % 4][65:66, :], in_=ctr[:])

                # ctx(hC) epilogue: store
                if doC:
                    _label(f"ostore:h{hC}")
                    nc.sync.dma_start(out=out_r[:, :, hC, :], in_=stC["ost"])
                    del state[hC]

    nc.compile()
    return nc


_module_cache = {}


def _get_module(T, H, NH):
    key = (T, H, NH)
    if key not in _module_cache:
        _module_cache[key] = build_module(T, H, NH)
    return _module_cache[key]


def run_sharded(hidden_states, attention_mask, w_qkv, b_qkv, trace=False):
    B, T, H = hidden_states.shape
    NH = H // HD
    nc = _get_module(T, H, NH)

    w_np = np.ascontiguousarray(w_qkv.astype(np.float32))
    b_np = np.asarray(b_qkv, dtype=np.float32)
    # qkb[h*128 + p] = b[h*192 + p]  (q bias 0:64, k bias 64:128 per head)
    qkb = np.empty((1, 128 * NH), np.float32)
    vbat = np.ones((1, (HD + 1) * NH), np.float32)
    for h in range(NH):
        qkb[0, h * 128 : (h + 1) * 128] = b_np[h * 3 * HD : h * 3 * HD + 128]
        vbat[0, h * (HD + 1) : h * (HD + 1) + HD] = b_np[
            h * 3 * HD + 2 * HD : h * 3 * HD + 3 * HD
        ]
    NHT = H // 128
    # wv_pre[p, ht, h, 0:64] = w[ht*128+p, h*192+128 : h*192+192]; slot 64 = 0
    wv4 = w_np.reshape(NHT, 128, NH, 3, HD)[:, :, :, 2, :]   # [ht, p, h, d]
    wv_pre = np.zeros((NHT, 128, NH, HD + 1), np.float32)
    wv_pre[:, :, :, 0:HD] = wv4
    wv_pre = np.ascontiguousarray(
        wv_pre.transpose(1, 0, 2, 3).reshape(128, NHT * NH * (HD + 1))
    )
    ones_row = np.ones((1, T), np.float32)
    neg_row = np.full((1, T), -1.0, np.float32)
    ident = np.eye(128, dtype=np.float32)

    in_maps = []
    for b in range(B):
        m = np.asarray(attention_mask[b]).reshape(-1).astype(np.float32)
        in_maps.append(
            dict(
                hidden=np.ascontiguousarray(hidden_states[b].astype(np.float32)),
                w=w_np,
                mask_neg=(m * np.float32(-10000.0)).reshape(1, T),
                ones_row=ones_row,
                neg_row=neg_row,
                qkb=qkb,
                vbat=vbat,
                wv_pre=wv_pre,
                ident_r=ident,
            )
        )
    res = run_bass_kernel_spmd(nc, in_maps, core_ids=list(range(B)), trace=trace)
    return np.stack([res.results[b]["out"] for b in range(B)]), res


def kernel(hidden_states, attention_mask, w_qkv, b_qkv):
    out, _ = run_sharded(
        np.asarray(hidden_states),
        np.asarray(attention_mask),
        np.asarray(w_qkv),
        np.asarray(b_qkv),
    )
    return out.astype(np.float32)
